# revision 66
# baseline (speedup 1.0000x reference)
"""BERT self-attention (B=4, S=2048, E=768, H=12) on 8 TRN2 NeuronCores.

Sharding: (batch, head-half) — core c handles batch c//2, heads 6*(c%2)..+6.
Each core is fully independent (no collectives).

Host-side prep (in kernel()): per-core shard slicing plus layout/precision
prep — hidden/W transposed to put the contraction dim on partitions, Wq/bq
pre-scaled by A16/sqrt(D), masks folded and pre-transformed on the host
(ET = exp(dm+am) bf16 for the ACT-path k-chunks, MS = A16*(dm+am)+B16 f32
for the Schraudolph-path chunks, both packed qq-major so the DMA streams
the first attention block's slice first), bv replicated across partitions.

Host-side POST: the device ships UNNORMALIZED ctx^T tiles with the softmax
denominator as row 64 (bf16); the host does the divide and the final
[j,qq,d,h,q] -> [s,e] transpose.  This removes the XBAR transposes,
reciprocals and the final broadcast-multiplies from the device entirely.

Device-side structure (per core):
  - projections (bf16): qT,kT in [o,m] layout; v in [m,o] layout augmented
    with a ones column per head (softmax denominators via the PV matmul);
    the v bias is folded into the PSUM-evacuation add.
  - scores^T[k,q] = kT.T @ qT, two heads row-packed per PE pass (d=64 at
    tile_position (0,0)/(64,0) -> the pair runs concurrently) into one f32
    PSUM tile [128, 1024], pre-scaled by A16 = 2^7/ln2.
  - softmax exp splits across two engines: 12/16 k-chunks: ACT exp
    (scale=1/A16) -> bf16, then DVE multiply by ET at bf16 2x; 4/16
    k-chunks use a one-op DVE Schraudolph: int16(S + MS) bit-cast as bf16
    IS exp(s+m) to ~1.8% rms.
  - PV: ctx_u^T[65,q] = v_aug.T @ prod accumulated over 16 k-chunks in
    PSUM; row 64 is the softmax denominator.
  - ctx_u^T evacuated PSUM->SBUF bf16 (one head on ACT Copy, one on DVE),
    then DMA'd straight out (bf16) — normalization happens on the host.

Startup: xT DMA'd in 4 column-chunks on the sync queue, the W's on the
scalar queue (3rd DMA queue), masks qq-sliced on the gpsimd queue, so the
oc=0 projections start ~2-3us in and stage D starts as soon as q/k oc=0 +
the first vaug chunks exist.  v-proj and the oc=1/2 projections are
spliced into stage-D block boundaries (transient 1-bank PSUM pools);
their bias-adds ride the Scalar engine (activation Copy with per-partition
bias — same ACT table set as Exp, no table-switch cost).
"""

import sys

if "/opt/trn_rl_repo" not in sys.path:
    sys.path.insert(0, "/opt/trn_rl_repo")

from contextlib import ExitStack

import ml_dtypes
import numpy as np

import concourse.bass as bass
import concourse.tile as tile
from concourse import bacc, mybir
from concourse.bass_utils import run_bass_kernel_spmd

B, S, E, H = 4, 2048, 768, 12
D = 64
N_CORES = 8
HPC = 6            # heads per core
EC = HPC * D       # 384 embedding cols per core
NIC = E // 128     # 6 contraction chunks
NOC = EC // 128    # 3 output chunks (= head pairs)
NKC = S // 128     # 16 k chunks
QW = 512           # q tile width
NQQ = S // QW      # 4 q chunks

F32 = mybir.dt.float32
BF16 = mybir.dt.bfloat16
I16 = mybir.dt.int16
Exp = mybir.ActivationFunctionType.Exp
Copy = mybir.ActivationFunctionType.Copy
Identity = mybir.ActivationFunctionType.Identity

# Schraudolph-in-int16: exp(x) ~ bitcast_bf16(int16(A16*x + B16)).  Scores
# arrive in PSUM pre-scaled by A16 (folded into Wq/bq); the ACT path undoes
# it with activation(scale=1/A16).  C16 calibrated for round-to-nearest.
A16 = float(2.0**7 / np.log(2.0))
C16 = 7.3
B16 = float(127 * 2**7 - C16)
# k-chunk split: 5 chunks -> DVE (Schraudolph) path, 11 -> ACT exp path
DVE_KC = [3, 6, 9, 12, 15]
ACT_KC = [kc for kc in range(NKC) if kc not in DVE_KC]
ET_IDX = {kc: i for i, kc in enumerate(ACT_KC)}
MS_IDX = {kc: i for i, kc in enumerate(DVE_KC)}
NACT = len(ACT_KC)
NMS = len(DVE_KC)


def _widen_last(ap, n: int):
    """Extend a unit-stride last dim to n elements (spills into the tile)."""
    assert ap.ap[-1][0] == 1
    return bass.AP(tensor=ap.tensor, offset=ap.offset, ap=[*ap.ap[:-1], [1, n]])


def _emit(ctx: ExitStack, tc: tile.TileContext, h):
    nc = tc.nc

    persist = ctx.enter_context(tc.tile_pool(name="persist", bufs=1))
    consts = ctx.enter_context(tc.tile_pool(name="consts", bufs=1))

    # ---- constants (tiny, gpsimd queue) ----
    bq_sb = consts.tile([128, NOC], F32)
    nc.gpsimd.dma_start(out=bq_sb[:], in_=h["bq"].ap())
    bk_sb = consts.tile([128, NOC], F32)
    nc.gpsimd.dma_start(out=bk_sb[:], in_=h["bk"].ap())
    bv_sb = consts.tile([128, EC], BF16)
    nc.gpsimd.dma_start(out=bv_sb[:], in_=h["bv"].ap())
    ones1 = consts.tile([1, 1], BF16)
    nc.vector.memset(ones1[:], 1.0)
    scratch1 = consts.tile([1, 1], BF16)
    # dummy exp at t~0: pulls the ACT exp-table load off the critical path
    nc.scalar.activation(scratch1[:], ones1[0:1, 0:1], Exp)

    # ---- persistent activations ----
    qT = persist.tile([128, NOC, S], BF16)        # [o%128, o-chunk, m]
    kT = persist.tile([128, NOC, S], BF16)
    # noqa: layout note — all DMA sources below are host-packed so every
    # per-partition run is contiguous (1.5-12KB packets, not 1KB strided)
    # [m%128, m-chunk, head, d|one|pad]: one zero pad head-slot at the end so
    # a 128-wide stationary AP (NumWeights==128 -> fast weight load) can spill
    # into the next slot for every real head
    vaug = persist.tile([128, NKC, HPC + 1, D + 4], BF16)
    ET = persist.tile([128, NACT, NQQ, QW], BF16)  # exp(mask), ACT-path chunks
    MS = persist.tile([128, NMS, NQQ, QW], F32)    # A16*mask + B16, DVE path

    # only the pad/ones regions need zeroing (v parts are fully overwritten)
    nc.vector.memset(vaug[:, :, :, D : D + 4], 0.0)
    nc.vector.memset(vaug[:, :, HPC, 0:D], 0.0)
    nc.vector.memset(vaug[:, :, 0:HPC, D : D + 1], 1.0)

    # stage-D pools open first so their SBUF/PSUM does not overlap the
    # projection pools (avoids release-chain stalls at the phase boundary).
    # PSUM budget: S_t 3x[128,1024] = 6 banks, ctx = 2 (proj pools
    # time-share the ctx banks: bufs=2 up front, transient ppx in stage D)
    sps = ctx.enter_context(tc.tile_pool(name="s_psum", bufs=3, space="PSUM"))
    exw = ctx.enter_context(tc.tile_pool(name="exw", bufs=4))
    # pr/pri need enough depth for the PV backlog (PVs for kc 0..6 flush
    # together when the ctx pool opens; at most 2 of those are Schraudolph)
    prw = ctx.enter_context(tc.tile_pool(name="prw", bufs=8))
    priw = ctx.enter_context(tc.tile_pool(name="priw", bufs=4))
    owork = ctx.enter_context(tc.tile_pool(name="owork", bufs=3))

    # ---- stages A+B: load + projections ----
    sab = ctx.enter_context(tc.tile_pool(name="stageAB", bufs=1))
    ppscm = tc.tile_pool(name="proj_psum", bufs=2, space="PSUM")
    pps0 = ppscm.__enter__()
    xTb = sab.tile([128, NIC, S], BF16)
    wqb = sab.tile([128, NOC, NIC, 128], BF16)
    wkb = sab.tile([128, NOC, NIC, 128], BF16)
    wvb = sab.tile([128, NIC, EC], BF16)

    # xT (bf16): 4 column-chunks split across the sync and gpsimd queues ->
    # first proj group starts after ~1/4 of the transfer
    for mq in range(NQQ):
        eng = nc.sync if mq % 2 == 0 else nc.gpsimd
        eng.dma_start(
            out=xTb[:, :, mq * QW : (mq + 1) * QW],
            in_=h["xT"].ap()[mq],
        )
    # W (bf16): scalar queue (3rd DMA queue), dependency-first order:
    # wq/wk oc=0 slabs, then wv (needed for v-proj), then the oc=1/2 slabs
    nc.scalar.dma_start(out=wqb[:, 0], in_=h["wqT"].ap()[0])
    nc.scalar.dma_start(out=wkb[:, 0], in_=h["wkT"].ap()[0])
    nc.scalar.dma_start(out=wvb[:], in_=h["wvT"].ap())
    for oc in range(1, NOC):
        nc.scalar.dma_start(out=wqb[:, oc], in_=h["wqT"].ap()[oc])
        nc.scalar.dma_start(out=wkb[:, oc], in_=h["wkT"].ap()[oc])

    # ---- stage C: masks (host-prepped qq-major so the first block's slice
    # lands first).  ET on gpsimd (behind xT mq1/3), MS on sync (behind
    # xT mq0/2). ----
    for qq in range(NQQ):
        nc.gpsimd.dma_start(out=ET[:, :, qq, :], in_=h["etT"].ap()[qq])
        nc.sync.dma_start(out=MS[:, :, qq, :], in_=h["msT"].ap()[qq])

    def proj_qk_group(pool, dst, wtb, bias, oc, mq, bias_eng):
        ps = pool.tile([128, QW], F32, tag="pp", name="pp_ps")
        for ic in range(NIC):
            nc.tensor.matmul(
                ps[:],
                wtb[:, oc, ic, :],
                xTb[:, ic, mq * QW : (mq + 1) * QW],
                start=(ic == 0),
                stop=(ic == NIC - 1),
            )
        dstap = dst[:, oc, mq * QW : (mq + 1) * QW]
        if bias_eng == "act":
            # Identity with per-partition bias rides the Scalar engine; same
            # ACT table set as Exp so no table-switch cost
            nc.scalar.activation(dstap, ps[:], Identity, bias=bias[:, oc : oc + 1])
        else:
            nc.vector.tensor_scalar_add(dstap, ps[:], bias[:, oc : oc + 1])

    def proj_v(mc, pool, evac_eng):
        vps_full = pool.tile([128, QW], F32, tag="pp", name="v_ps")
        vps = vps_full[:, 0:EC]
        for ic in range(NIC):
            nc.tensor.matmul(
                vps[:],
                xTb[:, ic, mc * 128 : (mc + 1) * 128],
                wvb[:, ic, :],
                start=(ic == 0),
                stop=(ic == NIC - 1),
            )
        # bias folded into the PSUM evacuation (bv pre-replicated on host)
        nc.vector.tensor_tensor(
            vaug[:, mc, 0:HPC, 0:D],
            vps[:].rearrange("p (h d) -> p h d", h=HPC),
            bv_sb[:].rearrange("p (h d) -> p h d", h=HPC),
            op=mybir.AluOpType.add,
        )

    # oc=0 projections + v-proj chunks 0..6 run up front (DVE/ACT are idle
    # here), interleaved by xT chunk so the PE starts as soon as chunk mq
    # lands; v-proj 7..15 streams inside the first attention block's
    # k-loop; oc=1/2 groups are spliced into later block boundaries
    for mq in range(NQQ):
        proj_qk_group(pps0, qT, wqb, bq_sb, 0, mq, "dve")
        proj_qk_group(pps0, kT, wkb, bk_sb, 0, mq, "dve")
        for mc in range(4 * mq, 4 * mq + 4):
            proj_v(mc, pps0, "dve")
    # oc=1/2 projection groups: 5 run up front (still overlapped with the
    # mask DMAs), 11 are spliced 2-per-boundary into stage D
    proj_feed = []
    for oc in range(1, NOC):
        for mq in range(NQQ):
            proj_feed.append((qT, wqb, bq_sb, oc, mq))
            proj_feed.append((kT, wkb, bk_sb, oc, mq))
    for _ in range(5):
        proj_qk_group(pps0, *proj_feed.pop(0), "dve")
    ppscm.__exit__(None, None, None)

    # ---- stage D: attention ----
    # The chunk tail is split: exp/mult (em) is emitted at lag-1 so the
    # Scalar queue streams exps uninterrupted across block boundaries; the
    # PV matmuls are backlogged until the ctx pool opens at kc==7 (after
    # the boundary proj splice at kc==2 and its deferred bias-adds at
    # kc==6 — ppx scope spans kc2..kc6, lexically before the ctx scope so
    # the static PSUM allocator time-shares the same banks race-free).
    def em(S_t, kc, qq):
        if kc in MS_IDX:
            # Schraudolph path: pr = bitcast_bf16(int16(S + (A16*m + B16)))
            pri = priw.tile([128, 2 * QW], I16, tag="pri")
            ms_ap = MS[:, MS_IDX[kc], qq, :]
            ms_b = bass.AP(
                tensor=ms_ap.tensor, offset=ms_ap.offset,
                ap=[ms_ap.ap[0], [0, 2], *ms_ap.ap[1:]],
            )
            nc.vector.tensor_tensor(
                pri[:].rearrange("p (g q) -> p g q", g=2),
                S_t[:].rearrange("p (g q) -> p g q", g=2),
                ms_b,
                op=mybir.AluOpType.add,
            )

            def mov(h0):
                return pri[:, h0 * QW : (h0 + 1) * QW].bitcast(BF16)
        else:
            ex = exw.tile([128, 2 * QW], BF16, tag="ex")
            nc.scalar.activation(ex[:], S_t[:], Exp, scale=1.0 / A16)
            pr = prw.tile([128, 2 * QW], BF16, tag="pr")
            et_ap = ET[:, ET_IDX[kc], qq, :]
            et_b = bass.AP(
                tensor=et_ap.tensor, offset=et_ap.offset,
                ap=[et_ap.ap[0], [0, 2], *et_ap.ap[1:]],
            )
            nc.vector.tensor_tensor(
                pr[:].rearrange("p (g q) -> p g q", g=2),
                ex[:].rearrange("p (g q) -> p g q", g=2),
                et_b,
                op=mybir.AluOpType.mult,
            )

            def mov(h0):
                return pr[:, h0 * QW : (h0 + 1) * QW]

        return mov

    def pv(mov, kc, j, ctxs):
        st, sp = (kc == 0), (kc == NKC - 1)
        ctxA, ctxB = ctxs
        # 128-wide stationary APs (spilling into the next head slot / the
        # zero pad slot) -> NumWeights==128 -> fast weight load.  Output
        # rows 65..127 accumulate next-head partials; only 0..64 are read.
        nc.tensor.matmul(
            ctxA[:], _widen_last(vaug[:, kc, 2 * j, 0:1], 128),
            mov(0), start=st, stop=sp,
        )
        nc.tensor.matmul(
            ctxB[:], _widen_last(vaug[:, kc, 2 * j + 1, 0:1], 128),
            mov(1), start=st, stop=sp,
        )

    # The previous block's epilogue (evacs, ctx close, out-DMA) flushes at
    # kc==3 of the NEXT block — after a few exps are queued, when PV(15) is
    # long done, so neither the ACT nor the DVE queue stalls on it.  PSUM
    # timeline per block: kc0-2 prev-ctx still held (6+2), kc4-7 ppx (6+2),
    # kc8+ own ctx (6+2) — all 8 banks busy the whole time.
    pending = []

    def flush_epilogue():
        if not pending:
            return
        ctxs, cps_cm, csb, j, qq = pending.pop()
        # evacuate PSUM accumulators to SBUF bf16, split across ACT and DVE
        nc.scalar.activation(csb[:, 0, :], ctxs[0][0:65, :], Copy)
        nc.vector.tensor_copy(csb[:, 1, :], ctxs[1][0:65, :])
        cps_cm.__exit__(None, None, None)
        # unnormalized ctx^T + denominator row straight out (bf16); the
        # host divides and transposes
        nc.sync.dma_start(out=h["out"].ap()[j, qq], in_=csb[:])

    def attn_block(qq, j, v_feed=None, ctx_at=7):
        qs = slice(qq * QW, (qq + 1) * QW)
        csb = owork.tile([65, 2, QW], BF16, tag="csb")
        cps_cm = None
        ctxs = None
        ppx_cm = None
        pend_bias = []
        pend_pv = []
        prev = None
        for kc in range(NKC):
            ks = slice(kc * 128, (kc + 1) * 128)
            S_t = sps.tile([128, 2 * QW], F32, tag="S")
            nc.tensor.matmul(
                S_t[:, 0:QW], kT[0:64, j, ks], qT[0:64, j, qs],
                start=True, stop=True, tile_position=(0, 0),
            )
            nc.tensor.matmul(
                S_t[:, QW : 2 * QW], kT[64:128, j, ks], qT[64:128, j, qs],
                start=True, stop=True, tile_position=(64, 0),
            )
            if v_feed and kc < ctx_at:
                # first block: v-proj chunks stream inside the k-loop in a
                # transient 1-bank pool (the ctx banks are still free)
                mc = v_feed.pop(0)
                with tc.tile_pool(name="vpx", bufs=1, space="PSUM") as vpx:
                    proj_v(mc, vpx, "act" if kc % 2 else "dve")
            if kc == 2 and not v_feed and proj_feed:
                # boundary proj splice: matmuls only; bias-adds deferred to
                # kc==6 (they'd stall their queue waiting on these matmuls)
                ppx_cm = tc.tile_pool(name="ppx", bufs=2, space="PSUM")
                ppx = ppx_cm.__enter__()
                for _ in range(2):
                    if proj_feed:
                        dst, wtb, bias, oc, mq = proj_feed.pop(0)
                        ps = ppx.tile([128, QW], F32, tag="pp", name="pp_ps")
                        for ic in range(NIC):
                            nc.tensor.matmul(
                                ps[:],
                                wtb[:, oc, ic, :],
                                xTb[:, ic, mq * QW : (mq + 1) * QW],
                                start=(ic == 0),
                                stop=(ic == NIC - 1),
                            )
                        pend_bias.append((dst, bias, oc, mq, ps))
            if kc == 6:
                while pend_bias:
                    dst, bias, oc, mq, ps = pend_bias.pop(0)
                    nc.scalar.activation(
                        dst[:, oc, mq * QW : (mq + 1) * QW], ps[:],
                        Identity, bias=bias[:, oc : oc + 1],
                    )
                if ppx_cm is not None:
                    ppx_cm.__exit__(None, None, None)
                    ppx_cm = None
            if kc == ctx_at:
                cps_cm = tc.tile_pool(name="ctxp", bufs=1, space="PSUM")
                cps = cps_cm.__enter__()
                ctxs = tuple(
                    cps.tile([128, QW], F32, tag=f"c{i}", name=f"ctx{i}")
                    for i in range(2)
                )
                for m, pkc in pend_pv:
                    pv(m, pkc, j, ctxs)
                pend_pv.clear()
            if prev is not None:
                m = em(prev[0], prev[1], qq)
                if ctxs is None:
                    pend_pv.append((m, prev[1]))
                else:
                    pv(m, prev[1], j, ctxs)
            prev = (S_t, kc)
        m = em(prev[0], prev[1], qq)
        pv(m, prev[1], j, ctxs)
        pending.append((ctxs, cps_cm, csb, j, qq))
        flush_epilogue()

    # j-major order: all q-chunks of a head pair before the next pair, so the
    # spliced oc=1/2 projections land before j=1/j=2 need them.  The first
    # block streams v-proj chunks 7..15 and defers its ctx pool to kc==9.
    for j in range(NOC):
        for qq in range(NQQ):
            attn_block(qq, j)
    flush_epilogue()


def build():
    nc = bacc.Bacc("TRN2", target_bir_lowering=False, debug=False, num_devices=N_CORES)
    h = {
        "xT": nc.dram_tensor("xT", [NQQ, 128, NIC, QW], BF16, kind="ExternalInput"),
        "wqT": nc.dram_tensor("wqT", [NOC, 128, NIC, 128], BF16, kind="ExternalInput"),
        "wkT": nc.dram_tensor("wkT", [NOC, 128, NIC, 128], BF16, kind="ExternalInput"),
        "wvT": nc.dram_tensor("wvT", [128, NIC, EC], BF16, kind="ExternalInput"),
        "bq": nc.dram_tensor("bq", [128, NOC], F32, kind="ExternalInput"),
        "bk": nc.dram_tensor("bk", [128, NOC], F32, kind="ExternalInput"),
        "bv": nc.dram_tensor("bv", [128, EC], BF16, kind="ExternalInput"),
        "etT": nc.dram_tensor(
            "etT", [NQQ, 128, NACT, QW], BF16, kind="ExternalInput"
        ),
        "msT": nc.dram_tensor(
            "msT", [NQQ, 128, NMS, QW], F32, kind="ExternalInput"
        ),
        "out": nc.dram_tensor("out", [NOC, NQQ, 65, 2, QW], BF16, kind="ExternalOutput"),
    }
    with tile.TileContext(nc) as tc:
        with ExitStack() as ctx:
            _emit(ctx, tc, h)
    nc.compile()
    return nc


def prep_in_maps(inputs):
    hs = np.asarray(inputs["hidden_states"], dtype=np.float32)
    am = np.asarray(inputs["attention_mask"], dtype=np.float32)
    dm = np.asarray(inputs["domain_attn_mask"], dtype=np.float32)
    Wq = np.asarray(inputs["Wq"], dtype=np.float32)
    bq = np.asarray(inputs["bq"], dtype=np.float32)
    Wk = np.asarray(inputs["Wk"], dtype=np.float32)
    bk = np.asarray(inputs["bk"], dtype=np.float32)
    Wv = np.asarray(inputs["Wv"], dtype=np.float32)
    bv = np.asarray(inputs["bv"], dtype=np.float32)

    qscale = 0.125 * A16
    in_maps = []
    mask_cache = {}
    for c in range(N_CORES):
        b = c // 2
        if b not in mask_cache:
            mfull = dm[b, 0].T + am[b, 0, 0, :, None]  # [k, q]
            mc = mfull.reshape(NKC, 128, NQQ, QW)
            # [NQQ, 128, NACT, QW] bf16, qq-major, p-major within a slab so
            # each partition's DMA run is contiguous
            et = np.ascontiguousarray(
                np.exp(mc[ACT_KC]).transpose(2, 1, 0, 3)
            ).astype(ml_dtypes.bfloat16)
            ms = np.ascontiguousarray(
                (A16 * mc[DVE_KC] + B16).transpose(2, 1, 0, 3)
            ).astype(np.float32)
            # xT [NQQ, 128, NIC, QW]
            xt = np.ascontiguousarray(
                hs[b].T.reshape(NIC, 128, NQQ, QW).transpose(2, 1, 0, 3)
            ).astype(ml_dtypes.bfloat16)
            mask_cache[b] = (et, ms, xt)
        et, ms, xt = mask_cache[b]
        e0 = (c % 2) * EC
        sl = slice(e0, e0 + EC)
        in_maps.append(
            {
                "xT": xt,
                "wqT": np.ascontiguousarray(
                    (Wq[sl, :].T * qscale)
                    .reshape(NIC, 128, NOC, 128)
                    .transpose(2, 1, 0, 3)
                ).astype(ml_dtypes.bfloat16),
                "wkT": np.ascontiguousarray(
                    Wk[sl, :].T.reshape(NIC, 128, NOC, 128).transpose(2, 1, 0, 3)
                ).astype(ml_dtypes.bfloat16),
                "wvT": np.ascontiguousarray(
                    Wv[sl, :].T.reshape(NIC, 128, EC).transpose(1, 0, 2)
                ).astype(ml_dtypes.bfloat16),
                "bq": np.ascontiguousarray((bq[sl] * qscale).reshape(NOC, 128).T),
                "bk": np.ascontiguousarray(bk[sl].reshape(NOC, 128).T),
                "bv": np.ascontiguousarray(
                    np.broadcast_to(bv[sl].reshape(1, EC), (128, EC))
                ).astype(ml_dtypes.bfloat16),
                "etT": et,
                "msT": ms,
            }
        )
    return in_maps


def postprocess(r, bv_sl=None):
    """Device out tile [NOC, NQQ, 65, 2, QW] -> [S, EC]: divide by the
    denominator row, transpose (bv is already folded on-device)."""
    r = np.asarray(r, dtype=np.float32)
    num = r[:, :, 0:D, :, :]              # [j, qq, d, h, q]
    den = r[:, :, D : D + 1, :, :]        # [j, qq, 1, h, q]
    ctx = num / den
    # [j, qq, d, h, q] -> [qq, q, j, h, d] -> [S, EC]
    return ctx.transpose(1, 4, 0, 3, 2).reshape(S, EC)


_cached_nc = None


def run(inputs, trace=False):
    global _cached_nc
    if _cached_nc is None:
        _cached_nc = build()
    in_maps = prep_in_maps(inputs)
    res = run_bass_kernel_spmd(
        _cached_nc, in_maps, core_ids=list(range(N_CORES)), trace=trace
    )
    bv = np.asarray(inputs["bv"], dtype=np.float32)
    out = np.empty((B, S, E), dtype=np.float32)
    for c in range(N_CORES):
        b = c // 2
        e0 = (c % 2) * EC
        out[b, :, e0 : e0 + EC] = postprocess(
            res.results[c]["out"], bv[e0 : e0 + EC]
        )
    return out, res


def kernel(**inputs) -> np.ndarray:
    return run(inputs)[0]


# revision 68
# speedup vs baseline: 1.1900x; 1.1900x over previous
"""BERT self-attention (B=4, S=2048, E=768, H=12) on 8 TRN2 NeuronCores.

Sharding: (batch, head-half) — core c handles batch c//2, heads 6*(c%2)..+6.
Each core is fully independent (no collectives).

Host-side prep (in kernel()): per-core shard slicing plus layout/precision
prep — hidden/W transposed to put the contraction dim on partitions, Wq/bq
pre-scaled by A16/sqrt(D), masks folded and pre-transformed on the host
(ET = exp(dm+am) bf16 for the ACT-path k-chunks, MS = A16*(dm+am)+B16 f32
for the Schraudolph-path chunks, both packed qq-major so the DMA streams
the first attention block's slice first), bv replicated across partitions.

Host-side POST: the device ships UNNORMALIZED ctx^T tiles with the softmax
denominator as row 64 (bf16); the host does the divide and the final
[j,qq,d,h,q] -> [s,e] transpose.  This removes the XBAR transposes,
reciprocals and the final broadcast-multiplies from the device entirely.

Device-side structure (per core):
  - projections (bf16): qT,kT in [o,m] layout; v in [m,o] layout augmented
    with a ones column per head (softmax denominators via the PV matmul);
    the v bias is folded into the PSUM-evacuation add.
  - scores^T[k,q] = kT.T @ qT, two heads row-packed per PE pass (d=64 at
    tile_position (0,0)/(64,0) -> the pair runs concurrently) into one f32
    PSUM tile [128, 1024], pre-scaled by A16 = 2^7/ln2.
  - softmax exp splits across two engines: 12/16 k-chunks: ACT exp
    (scale=1/A16) -> bf16, then DVE multiply by ET at bf16 2x; 4/16
    k-chunks use a one-op DVE Schraudolph: int16(S + MS) bit-cast as bf16
    IS exp(s+m) to ~1.8% rms.
  - PV: ctx_u^T[65,q] = v_aug.T @ prod accumulated over 16 k-chunks in
    PSUM; row 64 is the softmax denominator.
  - ctx_u^T evacuated PSUM->SBUF bf16 (one head on ACT Copy, one on DVE),
    then DMA'd straight out (bf16) — normalization happens on the host.

Startup: xT DMA'd in 4 column-chunks on the sync queue, the W's on the
scalar queue (3rd DMA queue), masks qq-sliced on the gpsimd queue, so the
oc=0 projections start ~2-3us in and stage D starts as soon as q/k oc=0 +
the first vaug chunks exist.  v-proj and the oc=1/2 projections are
spliced into stage-D block boundaries (transient 1-bank PSUM pools);
their bias-adds ride the Scalar engine (activation Copy with per-partition
bias — same ACT table set as Exp, no table-switch cost).
"""

import sys

if "/opt/trn_rl_repo" not in sys.path:
    sys.path.insert(0, "/opt/trn_rl_repo")

from contextlib import ExitStack

import ml_dtypes
import numpy as np

import concourse.bass as bass
import concourse.tile as tile
from concourse import bacc, mybir
from concourse.bass_utils import run_bass_kernel_spmd

B, S, E, H = 4, 2048, 768, 12
D = 64
N_CORES = 8
HPC = 6            # heads per core
EC = HPC * D       # 384 embedding cols per core
NIC = E // 128     # 6 contraction chunks
NOC = EC // 128    # 3 output chunks (= head pairs)
NKC = S // 128     # 16 k chunks
QW = 512           # q tile width
NQQ = S // QW      # 4 q chunks

F32 = mybir.dt.float32
BF16 = mybir.dt.bfloat16
I16 = mybir.dt.int16
Exp = mybir.ActivationFunctionType.Exp
Copy = mybir.ActivationFunctionType.Copy
Identity = mybir.ActivationFunctionType.Identity

# Schraudolph-in-int16: exp(x) ~ bitcast_bf16(int16(A16*x + B16)).  Scores
# arrive in PSUM pre-scaled by A16 (folded into Wq/bq); the ACT path undoes
# it with activation(scale=1/A16).  C16 calibrated for round-to-nearest.
A16 = float(2.0**7 / np.log(2.0))
C16 = 7.3
B16 = float(127 * 2**7 - C16)
# k-chunk split: 5 chunks -> DVE (Schraudolph) path, 11 -> ACT exp path
DVE_KC = [3, 6, 9, 12, 15]
ACT_KC = [kc for kc in range(NKC) if kc not in DVE_KC]
ET_IDX = {kc: i for i, kc in enumerate(ACT_KC)}
MS_IDX = {kc: i for i, kc in enumerate(DVE_KC)}
NACT = len(ACT_KC)
NMS = len(DVE_KC)


def _widen_last(ap, n: int):
    """Extend a unit-stride last dim to n elements (spills into the tile)."""
    assert ap.ap[-1][0] == 1
    return bass.AP(tensor=ap.tensor, offset=ap.offset, ap=[*ap.ap[:-1], [1, n]])


def _emit(ctx: ExitStack, tc: tile.TileContext, h):
    nc = tc.nc

    persist = ctx.enter_context(tc.tile_pool(name="persist", bufs=1))
    consts = ctx.enter_context(tc.tile_pool(name="consts", bufs=1))

    # ---- constants (tiny, gpsimd queue) ----
    bq_sb = consts.tile([128, NOC], F32)
    nc.gpsimd.dma_start(out=bq_sb[:], in_=h["bq"].ap())
    bk_sb = consts.tile([128, NOC], F32)
    nc.gpsimd.dma_start(out=bk_sb[:], in_=h["bk"].ap())
    bv_sb = consts.tile([128, EC], BF16)
    nc.gpsimd.dma_start(out=bv_sb[:], in_=h["bv"].ap())
    ones1 = consts.tile([1, 1], BF16)
    nc.vector.memset(ones1[:], 1.0)
    scratch1 = consts.tile([1, 1], BF16)
    # dummy exp at t~0: pulls the ACT exp-table load off the critical path
    nc.scalar.activation(scratch1[:], ones1[0:1, 0:1], Exp)

    # ---- persistent activations ----
    qT = persist.tile([128, NOC, S], BF16)        # [o%128, o-chunk, m]
    kT = persist.tile([128, NOC, S], BF16)
    # noqa: layout note — all DMA sources below are host-packed so every
    # per-partition run is contiguous (1.5-12KB packets, not 1KB strided)
    # [m%128, m-chunk, head, d|one|pad]: one zero pad head-slot at the end so
    # a 128-wide stationary AP (NumWeights==128 -> fast weight load) can spill
    # into the next slot for every real head
    vaug = persist.tile([128, NKC, HPC + 1, D + 4], BF16)
    ET = persist.tile([128, NACT, NQQ, QW], BF16)  # exp(mask), ACT-path chunks
    MS = persist.tile([128, NMS, NQQ, QW], F32)    # A16*mask + B16, DVE path

    # only the pad/ones regions need zeroing (v parts are fully overwritten)
    nc.vector.memset(vaug[:, :, :, D : D + 4], 0.0)
    nc.vector.memset(vaug[:, :, HPC, 0:D], 0.0)
    nc.vector.memset(vaug[:, :, 0:HPC, D : D + 1], 1.0)

    # stage-D pools open first so their SBUF/PSUM does not overlap the
    # projection pools (avoids release-chain stalls at the phase boundary).
    # PSUM budget: S_t 3x[128,1024] = 6 banks, ctx = 2 (proj pools
    # time-share the ctx banks: bufs=2 up front, transient ppx in stage D)
    sps = ctx.enter_context(tc.tile_pool(name="s_psum", bufs=3, space="PSUM"))
    exw = ctx.enter_context(tc.tile_pool(name="exw", bufs=4))
    prw = ctx.enter_context(tc.tile_pool(name="prw", bufs=4))
    priw = ctx.enter_context(tc.tile_pool(name="priw", bufs=4))
    owork = ctx.enter_context(tc.tile_pool(name="owork", bufs=3))

    # ---- stages A+B: load + projections ----
    sab = ctx.enter_context(tc.tile_pool(name="stageAB", bufs=1))
    ppscm = tc.tile_pool(name="proj_psum", bufs=2, space="PSUM")
    pps0 = ppscm.__enter__()
    xTb = sab.tile([128, NIC, S], BF16)
    wqb = sab.tile([128, NOC, NIC, 128], BF16)
    wkb = sab.tile([128, NOC, NIC, 128], BF16)
    wvb = sab.tile([128, NIC, EC], BF16)

    # xT (bf16): 4 column-chunks split across the sync and gpsimd queues ->
    # first proj group starts after ~1/4 of the transfer
    for mq in range(NQQ):
        eng = nc.sync if mq % 2 == 0 else nc.gpsimd
        eng.dma_start(
            out=xTb[:, :, mq * QW : (mq + 1) * QW],
            in_=h["xT"].ap()[mq],
        )
    # W (bf16): scalar queue (3rd DMA queue), dependency-first order:
    # wq/wk oc=0 slabs, then wv (needed for v-proj), then the oc=1/2 slabs
    nc.scalar.dma_start(out=wqb[:, 0], in_=h["wqT"].ap()[0])
    nc.scalar.dma_start(out=wkb[:, 0], in_=h["wkT"].ap()[0])
    nc.scalar.dma_start(out=wvb[:], in_=h["wvT"].ap())
    for oc in range(1, NOC):
        nc.scalar.dma_start(out=wqb[:, oc], in_=h["wqT"].ap()[oc])
        nc.scalar.dma_start(out=wkb[:, oc], in_=h["wkT"].ap()[oc])

    # ---- stage C: masks (host-prepped qq-major so the first block's slice
    # lands first).  ET on gpsimd (behind xT mq1/3), MS on sync (behind
    # xT mq0/2). ----
    for qq in range(NQQ):
        nc.gpsimd.dma_start(out=ET[:, :, qq, :], in_=h["etT"].ap()[qq])
        nc.sync.dma_start(out=MS[:, :, qq, :], in_=h["msT"].ap()[qq])

    def proj_qk_group(pool, dst, wtb, bias, oc, mq, bias_eng):
        ps = pool.tile([128, QW], F32, tag="pp", name="pp_ps")
        for ic in range(NIC):
            nc.tensor.matmul(
                ps[:],
                wtb[:, oc, ic, :],
                xTb[:, ic, mq * QW : (mq + 1) * QW],
                start=(ic == 0),
                stop=(ic == NIC - 1),
            )
        dstap = dst[:, oc, mq * QW : (mq + 1) * QW]
        if bias_eng == "act":
            # Identity with per-partition bias rides the Scalar engine; same
            # ACT table set as Exp so no table-switch cost
            nc.scalar.activation(dstap, ps[:], Identity, bias=bias[:, oc : oc + 1])
        else:
            nc.vector.tensor_scalar_add(dstap, ps[:], bias[:, oc : oc + 1])

    def proj_v(mc, pool, evac_eng):
        vps_full = pool.tile([128, QW], F32, tag="pp", name="v_ps")
        vps = vps_full[:, 0:EC]
        for ic in range(NIC):
            nc.tensor.matmul(
                vps[:],
                xTb[:, ic, mc * 128 : (mc + 1) * 128],
                wvb[:, ic, :],
                start=(ic == 0),
                stop=(ic == NIC - 1),
            )
        # bias folded into the PSUM evacuation (bv pre-replicated on host)
        nc.vector.tensor_tensor(
            vaug[:, mc, 0:HPC, 0:D],
            vps[:].rearrange("p (h d) -> p h d", h=HPC),
            bv_sb[:].rearrange("p (h d) -> p h d", h=HPC),
            op=mybir.AluOpType.add,
        )

    # oc=0 projections + v-proj chunks 0..6 run up front (DVE/ACT are idle
    # here), interleaved by xT chunk so the PE starts as soon as chunk mq
    # lands; v-proj 7..15 streams inside the first attention block's
    # k-loop; oc=1/2 groups are spliced into later block boundaries
    for mq in range(NQQ):
        proj_qk_group(pps0, qT, wqb, bq_sb, 0, mq, "dve")
        proj_qk_group(pps0, kT, wkb, bk_sb, 0, mq, "dve")
        for mc in range(4 * mq, 4 * mq + 4):
            proj_v(mc, pps0, "dve")
    # oc=1/2 projection groups: 5 run up front (still overlapped with the
    # mask DMAs), 11 are spliced 2-per-boundary into stage D
    proj_feed = []
    for oc in range(1, NOC):
        for mq in range(NQQ):
            proj_feed.append((qT, wqb, bq_sb, oc, mq))
            proj_feed.append((kT, wkb, bk_sb, oc, mq))
    for _ in range(5):
        proj_qk_group(pps0, *proj_feed.pop(0), "dve")
    ppscm.__exit__(None, None, None)

    # ---- stage D: attention ----
    # The chunk tail is split: exp/mult (em) is emitted at lag-1 so the
    # Scalar queue streams exps uninterrupted across block boundaries; the
    # PV matmuls are backlogged until the ctx pool opens at kc==7 (after
    # the boundary proj splice at kc==2 and its deferred bias-adds at
    # kc==6 — ppx scope spans kc2..kc6, lexically before the ctx scope so
    # the static PSUM allocator time-shares the same banks race-free).
    def em(S_t, kc, qq):
        if kc in MS_IDX:
            # Schraudolph path: pr = bitcast_bf16(int16(S + (A16*m + B16)))
            pri = priw.tile([128, 2 * QW], I16, tag="pri")
            ms_ap = MS[:, MS_IDX[kc], qq, :]
            ms_b = bass.AP(
                tensor=ms_ap.tensor, offset=ms_ap.offset,
                ap=[ms_ap.ap[0], [0, 2], *ms_ap.ap[1:]],
            )
            nc.vector.tensor_tensor(
                pri[:].rearrange("p (g q) -> p g q", g=2),
                S_t[:].rearrange("p (g q) -> p g q", g=2),
                ms_b,
                op=mybir.AluOpType.add,
            )

            def mov(h0):
                return pri[:, h0 * QW : (h0 + 1) * QW].bitcast(BF16)
        else:
            ex = exw.tile([128, 2 * QW], BF16, tag="ex")
            nc.scalar.activation(ex[:], S_t[:], Exp, scale=1.0 / A16)
            pr = prw.tile([128, 2 * QW], BF16, tag="pr")
            et_ap = ET[:, ET_IDX[kc], qq, :]
            et_b = bass.AP(
                tensor=et_ap.tensor, offset=et_ap.offset,
                ap=[et_ap.ap[0], [0, 2], *et_ap.ap[1:]],
            )
            nc.vector.tensor_tensor(
                pr[:].rearrange("p (g q) -> p g q", g=2),
                ex[:].rearrange("p (g q) -> p g q", g=2),
                et_b,
                op=mybir.AluOpType.mult,
            )

            def mov(h0):
                return pr[:, h0 * QW : (h0 + 1) * QW]

        return mov

    def pv(mov, kc, j, ctxs):
        st, sp = (kc == 0), (kc == NKC - 1)
        ctxA, ctxB = ctxs
        # 128-wide stationary APs (spilling into the next head slot / the
        # zero pad slot) -> NumWeights==128 -> fast weight load.  Output
        # rows 65..127 accumulate next-head partials; only 0..64 are read.
        nc.tensor.matmul(
            ctxA[:], _widen_last(vaug[:, kc, 2 * j, 0:1], 128),
            mov(0), start=st, stop=sp,
        )
        nc.tensor.matmul(
            ctxB[:], _widen_last(vaug[:, kc, 2 * j + 1, 0:1], 128),
            mov(1), start=st, stop=sp,
        )

    def attn_block(qq, j):
        qs = slice(qq * QW, (qq + 1) * QW)
        csb = owork.tile([65, 2, QW], BF16, tag="csb")
        with tc.tile_pool(name="ctxp", bufs=1, space="PSUM") as cps:
            ctxs = tuple(
                cps.tile([128, QW], F32, tag=f"c{i}", name=f"ctx{i}")
                for i in range(2)
            )
            prev = None
            for kc in range(NKC):
                ks = slice(kc * 128, (kc + 1) * 128)
                S_t = sps.tile([128, 2 * QW], F32, tag="S")
                nc.tensor.matmul(
                    S_t[:, 0:QW], kT[0:64, j, ks], qT[0:64, j, qs],
                    start=True, stop=True, tile_position=(0, 0),
                )
                nc.tensor.matmul(
                    S_t[:, QW : 2 * QW], kT[64:128, j, ks], qT[64:128, j, qs],
                    start=True, stop=True, tile_position=(64, 0),
                )
                if prev is not None:
                    m = em(prev[0], prev[1], qq)
                    pv(m, prev[1], j, ctxs)
                prev = (S_t, kc)
            m = em(prev[0], prev[1], qq)
            pv(m, prev[1], j, ctxs)
            # evacuate PSUM accumulators to SBUF bf16: one head on the
            # Scalar engine, one on DVE (parallel)
            nc.scalar.activation(csb[:, 0, :], ctxs[0][0:65, :], Copy)
            nc.vector.tensor_copy(csb[:, 1, :], ctxs[1][0:65, :])
        # unnormalized ctx^T + denominator row straight out (bf16); the
        # host divides and transposes
        nc.sync.dma_start(out=h["out"].ap()[j, qq], in_=csb[:])

    # j-major order: all q-chunks of a head pair before the next pair, so the
    # spliced oc=1/2 projections land before j=1/j=2 need them
    for j in range(NOC):
        for qq in range(NQQ):
            attn_block(qq, j)
            # deferred projection groups (2 per boundary) in a transient
            # 1-bank PSUM pool (time-shares banks with ctxp)
            for _ in range(2):
                if proj_feed:
                    with tc.tile_pool(name="ppx", bufs=1, space="PSUM") as ppx:
                        proj_qk_group(ppx, *proj_feed.pop(0), "act")


def build():
    nc = bacc.Bacc("TRN2", target_bir_lowering=False, debug=False, num_devices=N_CORES)
    h = {
        "xT": nc.dram_tensor("xT", [NQQ, 128, NIC, QW], BF16, kind="ExternalInput"),
        "wqT": nc.dram_tensor("wqT", [NOC, 128, NIC, 128], BF16, kind="ExternalInput"),
        "wkT": nc.dram_tensor("wkT", [NOC, 128, NIC, 128], BF16, kind="ExternalInput"),
        "wvT": nc.dram_tensor("wvT", [128, NIC, EC], BF16, kind="ExternalInput"),
        "bq": nc.dram_tensor("bq", [128, NOC], F32, kind="ExternalInput"),
        "bk": nc.dram_tensor("bk", [128, NOC], F32, kind="ExternalInput"),
        "bv": nc.dram_tensor("bv", [128, EC], BF16, kind="ExternalInput"),
        "etT": nc.dram_tensor(
            "etT", [NQQ, 128, NACT, QW], BF16, kind="ExternalInput"
        ),
        "msT": nc.dram_tensor(
            "msT", [NQQ, 128, NMS, QW], F32, kind="ExternalInput"
        ),
        "out": nc.dram_tensor("out", [NOC, NQQ, 65, 2, QW], BF16, kind="ExternalOutput"),
    }
    with tile.TileContext(nc) as tc:
        with ExitStack() as ctx:
            _emit(ctx, tc, h)
    nc.compile()
    return nc


def prep_in_maps(inputs):
    hs = np.asarray(inputs["hidden_states"], dtype=np.float32)
    am = np.asarray(inputs["attention_mask"], dtype=np.float32)
    dm = np.asarray(inputs["domain_attn_mask"], dtype=np.float32)
    Wq = np.asarray(inputs["Wq"], dtype=np.float32)
    bq = np.asarray(inputs["bq"], dtype=np.float32)
    Wk = np.asarray(inputs["Wk"], dtype=np.float32)
    bk = np.asarray(inputs["bk"], dtype=np.float32)
    Wv = np.asarray(inputs["Wv"], dtype=np.float32)
    bv = np.asarray(inputs["bv"], dtype=np.float32)

    qscale = 0.125 * A16
    in_maps = []
    mask_cache = {}
    for c in range(N_CORES):
        b = c // 2
        if b not in mask_cache:
            mfull = dm[b, 0].T + am[b, 0, 0, :, None]  # [k, q]
            mc = mfull.reshape(NKC, 128, NQQ, QW)
            # [NQQ, 128, NACT, QW] bf16, qq-major, p-major within a slab so
            # each partition's DMA run is contiguous
            et = np.ascontiguousarray(
                np.exp(mc[ACT_KC]).transpose(2, 1, 0, 3)
            ).astype(ml_dtypes.bfloat16)
            ms = np.ascontiguousarray(
                (A16 * mc[DVE_KC] + B16).transpose(2, 1, 0, 3)
            ).astype(np.float32)
            # xT [NQQ, 128, NIC, QW]
            xt = np.ascontiguousarray(
                hs[b].T.reshape(NIC, 128, NQQ, QW).transpose(2, 1, 0, 3)
            ).astype(ml_dtypes.bfloat16)
            mask_cache[b] = (et, ms, xt)
        et, ms, xt = mask_cache[b]
        e0 = (c % 2) * EC
        sl = slice(e0, e0 + EC)
        in_maps.append(
            {
                "xT": xt,
                "wqT": np.ascontiguousarray(
                    (Wq[sl, :].T * qscale)
                    .reshape(NIC, 128, NOC, 128)
                    .transpose(2, 1, 0, 3)
                ).astype(ml_dtypes.bfloat16),
                "wkT": np.ascontiguousarray(
                    Wk[sl, :].T.reshape(NIC, 128, NOC, 128).transpose(2, 1, 0, 3)
                ).astype(ml_dtypes.bfloat16),
                "wvT": np.ascontiguousarray(
                    Wv[sl, :].T.reshape(NIC, 128, EC).transpose(1, 0, 2)
                ).astype(ml_dtypes.bfloat16),
                "bq": np.ascontiguousarray((bq[sl] * qscale).reshape(NOC, 128).T),
                "bk": np.ascontiguousarray(bk[sl].reshape(NOC, 128).T),
                "bv": np.ascontiguousarray(
                    np.broadcast_to(bv[sl].reshape(1, EC), (128, EC))
                ).astype(ml_dtypes.bfloat16),
                "etT": et,
                "msT": ms,
            }
        )
    return in_maps


def postprocess(r, bv_sl=None):
    """Device out tile [NOC, NQQ, 65, 2, QW] -> [S, EC]: divide by the
    denominator row, transpose (bv is already folded on-device)."""
    r = np.asarray(r, dtype=np.float32)
    num = r[:, :, 0:D, :, :]              # [j, qq, d, h, q]
    den = r[:, :, D : D + 1, :, :]        # [j, qq, 1, h, q]
    ctx = num / den
    # [j, qq, d, h, q] -> [qq, q, j, h, d] -> [S, EC]
    return ctx.transpose(1, 4, 0, 3, 2).reshape(S, EC)


_cached_nc = None


def run(inputs, trace=False):
    global _cached_nc
    if _cached_nc is None:
        _cached_nc = build()
    in_maps = prep_in_maps(inputs)
    res = run_bass_kernel_spmd(
        _cached_nc, in_maps, core_ids=list(range(N_CORES)), trace=trace
    )
    bv = np.asarray(inputs["bv"], dtype=np.float32)
    out = np.empty((B, S, E), dtype=np.float32)
    for c in range(N_CORES):
        b = c // 2
        e0 = (c % 2) * EC
        out[b, :, e0 : e0 + EC] = postprocess(
            res.results[c]["out"], bv[e0 : e0 + EC]
        )
    return out, res


def kernel(**inputs) -> np.ndarray:
    return run(inputs)[0]
